# revision 1
# baseline (speedup 1.0000x reference)
# Trainium2 Bass kernel for nn_BAKTTime (dense_transformer).
# Self-contained: hardcodes shapes B=2, L=2048, D=256, H=8, dk=32.
#
# Sharding: 8 cores, SPMD program. core i handles batch (i & 1) and query
# variant (i // 2); variant j computes the position-local pipeline (folded
# 5-tap conv + layernorm + capsule routing + banded time attention + fusion
# + qkv) over the causal prefix [0, A[j+1]) its queries need, then flash
# MHA over q in [A[j], A[j+1]).  No cross-core communication; the host
# stitches the disjoint query rows of the 8 partial outputs.
import numpy as np

B, L, D = 2, 2048, 256
H, DK = 8, 32
DECAY = 0.2
EPS_LN = 1e-12
NEG = -1e30
CW = 512
ABOUNDS = (0, 1152, 1536, 1792, 2048)


def _host_prep(inp):
    f32 = np.float32
    x = np.asarray(inp["x"], f32)
    c3w, c3b = np.asarray(inp["conv3_w"], f32), np.asarray(inp["conv3_b"], f32)
    c5w, c5b = np.asarray(inp["conv5_w"], f32), np.asarray(inp["conv5_b"], f32)
    b3 = np.asarray(inp["beta3"], f32).reshape(D)
    b5 = np.asarray(inp["beta5"], f32).reshape(D)
    cw = np.asarray(inp["combine_w"], f32)
    cwt = np.exp(cw - cw.max())
    cwt = (cwt / cwt.sum()).astype(f32)
    g3 = (cwt[0] * (1.0 - b3 ** 2)).astype(f32)
    g5 = (cwt[1] * (1.0 - b5 ** 2)).astype(f32)
    dscale = (1.0 + cwt[0] * b3 ** 2 + cwt[1] * b5 ** 2).astype(f32)
    W = np.zeros((5, D, D), f32)
    W[0] = g3[:, None] * c3w[:, :, 2] + g5[:, None] * c5w[:, :, 4] + np.diag(dscale)
    W[1] = g3[:, None] * c3w[:, :, 1] + g5[:, None] * c5w[:, :, 3]
    W[2] = g3[:, None] * c3w[:, :, 0] + g5[:, None] * c5w[:, :, 2]
    W[3] = g5[:, None] * c5w[:, :, 1]
    W[4] = g5[:, None] * c5w[:, :, 0]
    # lhsT layout [din_par, din_ch, tap, o_ch, dout]
    wtT = np.transpose(W, (0, 2, 1)).reshape(5, 2, 128, 2, 128)
    wtT = np.ascontiguousarray(np.transpose(wtT, (2, 1, 0, 3, 4)))
    beff = (g3 * c3b + g5 * c5b).reshape(2, 128).T.copy()            # [128, 2]
    dw3 = np.asarray(inp["dw3_w"], f32)[:, 0, :]
    dw5 = np.asarray(inp["dw5_w"], f32)[:, 0, :]
    c3l = np.zeros((5, D), f32)
    c5l = np.zeros((5, D), f32)
    for l in range(3):
        c3l[l] = dw3[:, 2 - l]
    for l in range(5):
        c5l[l] = dw5[:, 4 - l]
    pco, dco = c3l + c5l, c3l - c5l
    dwdiag = np.zeros((2, 5, 2, 128, 128), f32)
    for l in range(5):
        for o in range(2):
            dwdiag[0, l, o] = np.diag(pco[l, o * 128:(o + 1) * 128])
            dwdiag[1, l, o] = np.diag(dco[l, o * 128:(o + 1) * 128])
    dwdiag = np.ascontiguousarray(np.transpose(dwdiag, (3, 0, 1, 2, 4)))  # [128,2,5,2,128]
    col = lambda v: np.asarray(v, f32).reshape(2, 128).T.copy()      # [128, 2]
    fwT = np.asarray(inp["fusion_w"], f32).T.reshape(4, 128, 2, 128)
    fwT = np.ascontiguousarray(np.transpose(fwT, (1, 0, 2, 3)))      # [128,4,2,128]
    s = 1.0 / np.sqrt(DK)
    def sqT(w):
        t = np.asarray(w, f32).T.reshape(2, 128, 2, 128)
        return np.ascontiguousarray(np.transpose(t, (1, 0, 2, 3)))   # [128,2,2,128]
    def hvT(w):
        t = np.asarray(w, f32).T.reshape(2, 128, 256)
        return np.ascontiguousarray(np.transpose(t, (1, 0, 2)))      # [128,2,256]
    ii = np.arange(128)
    T128 = np.where(ii[None, :] >= ii[:, None], 0.0, NEG).astype(f32)
    dbias = np.zeros((1, 512), f32)
    for bb in range(4):
        dbias[0, bb * 128:(bb + 1) * 128] = DECAY * (ii + 128.0 * (bb - 2))
    xT = np.zeros((B, 128, 2, 4 + L), f32)
    xt_full = np.transpose(x, (0, 2, 1)).reshape(B, 2, 128, L)
    xT[:, :, :, 4:] = np.transpose(xt_full, (0, 2, 1, 3))
    shared = dict(
        wtT=wtT, beff=beff, dwdiag=dwdiag,
        lnw=col(inp["ln_w"]), lnb=col(inp["ln_b"]),
        fwT=fwT, fb=col(inp["fusion_b"]),
        wqT=sqT(np.asarray(inp["wq"], f32) * s), bq=col(np.asarray(inp["bq"], f32) * s),
        wkT=sqT(inp["wk"]), bk=col(inp["bk"]),
        wvT=hvT(inp["wv"]), bv_b=np.tile(np.asarray(inp["bv"], f32)[None, :], (128, 1)),
        woT=np.ascontiguousarray(np.asarray(inp["wo"], f32).T.reshape(8, 32, 256).transpose(1, 0, 2)),
        bo_b=np.tile(np.asarray(inp["bo"], f32)[None, :], (128, 1)),
        T128=T128, dbias=dbias, ones_row=np.ones((1, 512), f32),
        eye=np.eye(128, dtype=f32), ones128=np.ones((128, 128), f32),
    )
    return shared, xT


def _build(force_variant=None):
    import concourse.mybir as mybir
    import concourse.tile as tile
    from concourse import bacc

    F32, F32R = mybir.dt.float32, mybir.dt.float32r
    AF = mybir.ActivationFunctionType
    ALU = mybir.AluOpType

    nc = bacc.Bacc()

    def din(name, shape, dt=F32R):
        return nc.dram_tensor(name, list(shape), dt, kind="ExternalInput")

    xTd = din("xT", (128, 2, 4 + L))
    wtTd = din("wtT", (128, 2, 5, 2, 128))
    beffd = din("beff", (128, 2), F32)
    dwdiagd = din("dwdiag", (128, 2, 5, 2, 128))
    lnwd = din("lnw", (128, 2), F32)
    lnbd = din("lnb", (128, 2), F32)
    fwTd = din("fwT", (128, 4, 2, 128))
    fbd = din("fb", (128, 2), F32)
    wqTd = din("wqT", (128, 2, 2, 128))
    bqd = din("bq", (128, 2), F32)
    wkTd = din("wkT", (128, 2, 2, 128))
    bkd = din("bk", (128, 2), F32)
    wvTd = din("wvT", (128, 2, 256))
    bvbd = din("bv_b", (128, 256), F32)
    woTd = din("woT", (32, 8, 256))
    bobd = din("bo_b", (128, 256), F32)
    T128d = din("T128", (128, 128), F32)
    dbiasd = din("dbias", (1, 512))
    onesrd = din("ones_row", (1, 512))
    eyed = din("eye", (128, 128))
    ones128d = din("ones128", (128, 128))
    outd = nc.dram_tensor("out", [L, D], F32, kind="ExternalOutput")

    with tile.TileContext(nc) as tc:
        with tc.tile_pool(name="wpool", bufs=1) as wpool, \
             tc.tile_pool(name="ppool", bufs=1) as ppool, \
             tc.tile_pool(name="work", bufs=1) as work, \
             tc.tile_pool(name="wk3", bufs=2) as wk3, \
             tc.tile_pool(name="pspool", bufs=1, space="PSUM") as pspool:
            psO = pspool

            def load(name, dram, shape, dt=F32R):
                t = wpool.tile(list(shape), dt, name=name)
                nc.sync.dma_start(t[:], dram[:])
                return t

            wtT_sb = load("wtT_sb", wtTd, (128, 2, 5, 2, 128))
            dwd_sb = load("dwd_sb", dwdiagd, (128, 2, 5, 2, 128))
            fwT_sb = load("fwT_sb", fwTd, (128, 4, 2, 128))
            wqT_sb = load("wqT_sb", wqTd, (128, 2, 2, 128))
            wkT_sb = load("wkT_sb", wkTd, (128, 2, 2, 128))
            wvT_sb = load("wvT_sb", wvTd, (128, 2, 256))
            woT_sb = load("woT_sb", woTd, (32, 8, 256))
            beff_sb = load("beff_sb", beffd, (128, 2), F32)
            lnw_sb = load("lnw_sb", lnwd, (128, 2), F32)
            lnb_sb = load("lnb_sb", lnbd, (128, 2), F32)
            fb_sb = load("fb_sb", fbd, (128, 2), F32)
            bq_sb = load("bq_sb", bqd, (128, 2), F32)
            bk_sb = load("bk_sb", bkd, (128, 2), F32)
            bvb_sb = load("bvb_sb", bvbd, (128, 256), F32)
            bob_sb = load("bob_sb", bobd, (128, 256), F32)
            T_sb = load("T_sb", T128d, (128, 128), F32)
            dbias_sb = load("dbias_sb", dbiasd, (1, 512))
            onesr_sb = load("onesr_sb", onesrd, (1, 512))
            eye_sb = load("eye_sb", eyed, (128, 128))
            ones_sb = load("ones_sb", ones128d, (128, 128))

            zeros8 = wpool.tile([128, 8], F32, name="zeros8")
            nc.vector.memset(zeros8[:], 0.0)
            eps_sb = wpool.tile([128, 1], F32, name="eps_sb")
            nc.vector.memset(eps_sb[:], EPS_LN)
            tiny_sb = wpool.tile([128, 1], F32, name="tiny_sb")
            nc.vector.memset(tiny_sb[:], 1e-30)
            hT = ppool.tile([128, 2, 2, 4 + CW], F32R, name="hT")
            vT_s = ppool.tile([128, 2, 2, CW], F32R, name="vT_s")
            vT_t = ppool.tile([128, 2, 2, CW], F32R, name="vT_t")
            vn_s = ppool.tile([128, 2, 4, 256], F32R, name="vn_s")
            vn_t = ppool.tile([128, 2, 4, 256], F32R, name="vn_t")
            v_all = ppool.tile([128, L // 128, 256], F32R, name="v_all")
            qT_all = ppool.tile([128, 2, L], F32R, name="qT_all")
            kT_all = ppool.tile([128, 2, L], F32R, name="kT_all")
            for _o in range(2):
                for _p in range(2):
                    nc.vector.tensor_copy(out=hT[:, _o, _p, 0:4], in_=zeros8[:, 0:4])
            vT = {0: vT_s, 1: vT_t}
            vn = {0: vn_s, 1: vn_t}

            def emit_chunk(l0, w):
                par = (l0 // CW) & 1
                c0 = l0 % CW
                nblk = w // 128
                # ---- folded conv (transposed layout) ----
                xr = work.tile([128, 2, 4 + CW], F32R, tag="xr")
                nc.sync.dma_start(xr[:, :, 0:4 + w], xTd[:, :, l0:l0 + 4 + w])
                y = work.tile([128, 2, CW], F32R, tag="y_sb")
                sq = work.tile([128, 2, CW], F32R, tag="sq_sb")
                for o in range(2):
                    ps = pspool.tile([128, CW], F32, tag="T1")
                    for t in range(5):
                        for k in range(2):
                            nc.tensor.matmul(ps[:, 0:w], wtT_sb[:, k, t, o, :],
                                             xr[:, k, 4 - t:4 - t + w],
                                             start=(t == 0 and k == 0),
                                             stop=(t == 4 and k == 1))
                    nc.scalar.activation(y[:, o, 0:w], ps[:, 0:w], AF.Identity,
                                         bias=beff_sb[:, o:o + 1], scale=1.0)
                    nc.scalar.activation(sq[:, o, 0:w], ps[:, 0:w], AF.Square,
                                         bias=beff_sb[:, o:o + 1], scale=1.0)
                # ---- layernorm via replicated-moment matmuls ----
                msum = pspool.tile([128, CW], F32, tag="T2")
                msq = pspool.tile([128, CW], F32, tag="T3")
                for o in range(2):
                    nc.tensor.matmul(msum[:, 0:w], ones_sb[:], y[:, o, 0:w],
                                     start=(o == 0), stop=(o == 1))
                    nc.tensor.matmul(msq[:, 0:w], ones_sb[:], sq[:, o, 0:w],
                                     start=(o == 0), stop=(o == 1))
                mu = work.tile([128, CW], F32, tag="mu")
                nc.vector.tensor_scalar_mul(out=mu[:, 0:w], in0=msum[:, 0:w], scalar1=1.0 / D)
                mu2 = work.tile([128, CW], F32, tag="tmpA")
                nc.vector.tensor_mul(out=mu2[:, 0:w], in0=mu[:, 0:w], in1=mu[:, 0:w])
                var = work.tile([128, CW], F32, tag="tmpB")
                nc.vector.scalar_tensor_tensor(out=var[:, 0:w], in0=msq[:, 0:w],
                                               scalar=1.0 / D, in1=mu2[:, 0:w],
                                               op0=ALU.mult, op1=ALU.subtract)
                lnv = work.tile([128, CW], F32, tag="tmpA")
                nc.scalar.activation(lnv[:, 0:w], var[:, 0:w], AF.Ln, bias=eps_sb[:])
                rstd = work.tile([128, CW], F32, tag="tmpB")
                nc.scalar.activation(rstd[:, 0:w], lnv[:, 0:w], AF.Exp, scale=-0.5)
                for o in range(2):
                    t1 = work.tile([128, CW], F32, tag="tmpA")
                    nc.vector.tensor_sub(out=t1[:, 0:w], in0=y[:, o, 0:w], in1=mu[:, 0:w])
                    nc.vector.tensor_mul(out=t1[:, 0:w], in0=t1[:, 0:w], in1=rstd[:, 0:w])
                    nc.vector.tensor_scalar(out=hT[:, o, par, 4 + c0:4 + c0 + w],
                                            in0=t1[:, 0:w],
                                            scalar1=lnw_sb[:, o:o + 1],
                                            scalar2=lnb_sb[:, o:o + 1],
                                            op0=ALU.mult, op1=ALU.add)
                if c0 + w == CW:
                    nc.vector.tensor_copy(out=hT[:, :, 1 - par, 0:4],
                                          in_=hT[:, :, par, CW:CW + 4])
                # ---- trend taps: P = t3+t5, Dt = t3-t5 (diag matmuls) ----
                wsT = work.tile([128, 2, CW], F32R, tag="bufA")
                wtTt = work.tile([128, 2, CW], F32R, tag="bufB")
                dnT = work.tile([128, 2, CW], F32R, tag="bufC")
                for pd in range(2):
                    for o in range(2):
                        ps = pspool.tile([128, CW], F32, tag="T4")
                        for t in range(5):
                            nc.tensor.matmul(ps[:, 0:w], dwd_sb[:, pd, t, o, :],
                                             hT[:, o, par, 4 + c0 - t:4 + c0 - t + w],
                                             start=(t == 0), stop=(t == 4))
                        if pd == 0:
                            nc.vector.scalar_tensor_tensor(out=wsT[:, o, 0:w],
                                                           in0=hT[:, o, par, 4 + c0:4 + c0 + w],
                                                           scalar=2.0, in1=ps[:, 0:w],
                                                           op0=ALU.mult, op1=ALU.subtract)
                            nc.scalar.activation(wtTt[:, o, 0:w], ps[:, 0:w],
                                                 AF.Copy, bias=0.0, scale=1.0)
                        else:
                            nc.scalar.activation(dnT[:, o, 0:w], ps[:, 0:w],
                                                 AF.Copy, bias=0.0, scale=1.0)
                # ---- transpose routing inputs to [l, c] ----
                wsn = work.tile([128, 4, 256], F32, tag="sq_sb")
                wtn = work.tile([128, 4, 256], F32, tag="xr")
                dnn = work.tile([128, 4, 256], F32, tag="tmpA")
                for srct, dst in ((wsT, wsn), (wtTt, wtn), (dnT, dnn)):
                    for bi in range(nblk):
                        pst = pspool.tile([128, 2, 128], F32R, tag="T4")
                        for o in range(2):
                            nc.tensor.transpose(pst[:, o, :], srct[:, o, bi * 128:(bi + 1) * 128], eye_sb[:])
                        nc.vector.tensor_copy(out=dst[:, bi, :], in_=pst[:])
                # ---- dynamic routing (3 iters), seasonal then trend ----
                blk0 = c0 // 128
                for prob in range(2):
                    wn = wsn if prob == 0 else wtn
                    dsign = -1.0 if prob == 0 else 1.0
                    st = work.tile([128, 4, 256], F32, tag="y_sb")
                    g = work.tile([128, 16, 4], F32, tag="g")
                    diff = work.tile([128, 4], F32, tag="diff")
                    nc.vector.memset(g[:], 0.0)
                    nc.vector.memset(diff[:], 0.0)
                    scr = work.tile([128, 256], F32, tag="mu")
                    for it in range(3):
                        src_s = wn if it == 0 else st
                        for bi in range(nblk):
                            nc.vector.scalar_tensor_tensor(
                                out=scr[:], in0=src_s[:, bi, :], scalar=1.0,
                                in1=src_s[:, bi, :], op0=ALU.mult, op1=ALU.mult,
                                accum_out=g[:, 0, bi:bi + 1])
                        nc.scalar.activation(g[:, 1, 0:4], g[:, 0, 0:4], AF.Ln, bias=tiny_sb[:])
                        nc.scalar.activation(g[:, 2, 0:4], g[:, 1, 0:4], AF.Exp, scale=0.5)
                        nc.vector.tensor_scalar(out=g[:, 3, 0:4], in0=g[:, 0, 0:4],
                                                scalar1=0.25, scalar2=1.0,
                                                op0=ALU.mult, op1=ALU.add)
                        nc.vector.tensor_scalar(out=g[:, 4, 0:4], in0=g[:, 2, 0:4],
                                                scalar1=0.5, scalar2=1e-9,
                                                op0=ALU.mult, op1=ALU.add)
                        nc.vector.tensor_mul(out=g[:, 5, 0:4], in0=g[:, 3, 0:4], in1=g[:, 4, 0:4])
                        nc.vector.tensor_scalar_mul(out=g[:, 6, 0:4], in0=g[:, 0, 0:4], scalar1=0.125)
                        nc.vector.reciprocal_approx_accurate(out=g[:, 13, 0:4], in_=g[:, 5, 0:4],
                                                             scratch=g[:, 12, 0:4])
                        nc.vector.tensor_mul(out=g[:, 7, 0:4], in0=g[:, 6, 0:4], in1=g[:, 13, 0:4])
                        if it < 2:
                            for bi in range(nblk):
                                nc.vector.scalar_tensor_tensor(
                                    out=scr[:], in0=dnn[:, bi, :], scalar=1.0,
                                    in1=src_s[:, bi, :], op0=ALU.mult, op1=ALU.mult,
                                    accum_out=g[:, 8, bi:bi + 1])
                            nc.vector.tensor_mul(out=g[:, 9, 0:4], in0=g[:, 8, 0:4], in1=g[:, 7, 0:4])
                            if it == 0:
                                nc.vector.tensor_scalar_mul(out=diff[:, 0:4], in0=g[:, 9, 0:4], scalar1=dsign)
                            else:
                                nc.vector.scalar_tensor_tensor(out=diff[:, 0:4], in0=g[:, 9, 0:4],
                                                               scalar=dsign, in1=diff[:, 0:4],
                                                               op0=ALU.mult, op1=ALU.add)
                            nc.scalar.activation(g[:, 10, 0:4], diff[:, 0:4], AF.Exp, scale=-1.0)
                            nc.vector.tensor_scalar_add(out=g[:, 10, 0:4], in0=g[:, 10, 0:4], scalar1=1.0)
                            nc.vector.reciprocal_approx_accurate(out=g[:, 11, 0:4], in_=g[:, 10, 0:4], scratch=g[:, 12, 0:4])
                            nc.vector.tensor_scalar(out=g[:, 11, 0:4], in0=g[:, 11, 0:4],
                                                    scalar1=2.0 * dsign, scalar2=-1.0 * dsign,
                                                    op0=ALU.mult, op1=ALU.add)
                            for bi in range(nblk):
                                nc.vector.scalar_tensor_tensor(
                                    out=st[:, bi, :], in0=dnn[:, bi, :],
                                    scalar=g[:, 11, bi:bi + 1], in1=wn[:, bi, :],
                                    op0=ALU.mult, op1=ALU.add)
                        else:
                            for bi in range(nblk):
                                nc.vector.tensor_scalar_mul(
                                    out=vn[prob][:, par, blk0 + bi, :],
                                    in0=src_s[:, bi, :], scalar1=g[:, 7, bi:bi + 1])
                # ---- transpose v to vT ----
                for prob in range(2):
                    for bi in range(nblk):
                        pst = pspool.tile([128, 2, 128], F32R, tag="T4")
                        for o in range(2):
                            nc.tensor.transpose(pst[:, o, :], vn[prob][:, par, blk0 + bi, o * 128:(o + 1) * 128], eye_sb[:])
                        nc.vector.tensor_copy(out=vT[prob][:, :, par, c0 + bi * 128:c0 + (bi + 1) * 128], in_=pst[:])
                # ---- banded time attention ----
                sfT = work.tile([128, 2, CW], F32R, tag="bufA")
                tfT = work.tile([128, 2, CW], F32R, tag="bufB")
                for prob in range(2):
                    vTt, vnt = vT[prob], vn[prob]
                    dstT = sfT if prob == 0 else tfT
                    q0 = 0
                    while q0 < w:
                        qw = min(256, w - q0)
                        Q0 = l0 + q0
                        bbs = [bb for bb in range(4)
                               if Q0 + 128 * (bb - 2) >= 0 and 128 * (bb - 2) < qw]
                        Sps = pspool.tile([128, 4, 256], F32, tag="Sbig")
                        Pt = wk3.tile([128, 4, 256], F32R, tag="Pbuf")
                        zones = {}
                        for bb in bbs:
                            zones.setdefault(bb // 2, []).append(bb)
                        for z, zbbs in zones.items():
                            for bb in zbbs:
                                K0 = Q0 + 128 * (bb - 2)
                                kpar, kc = (K0 // CW) & 1, K0 % CW
                                lo = max(0, 128 * (bb - 2))
                                nc.tensor.matmul(Sps[:, bb, 0:qw],
                                                 dbias_sb[:, bb * 128:(bb + 1) * 128],
                                                 onesr_sb[:, 0:qw],
                                                 start=(bb == zbbs[0]), stop=False)
                                for o in range(2):
                                    nc.tensor.matmul(Sps[:, bb, lo:qw],
                                                     vTt[:, o, kpar, kc:kc + 128],
                                                     vTt[:, o, par, c0 + q0 + lo:c0 + q0 + qw],
                                                     start=False,
                                                     stop=(bb == zbbs[-1] and o == 1))
                            for bb in zbbs:
                                if bb >= 2:
                                    dlo = 128 * (bb - 2)
                                    dwd = min(qw, dlo + 128) - dlo
                                    nc.vector.tensor_add(out=Sps[:, bb, dlo:dlo + dwd],
                                                         in0=Sps[:, bb, dlo:dlo + dwd],
                                                         in1=T_sb[:, 0:dwd])
                            nc.scalar.activation(Pt[:, 2 * z:2 * z + len(zbbs), 0:qw],
                                                 Sps[:, 2 * z:2 * z + len(zbbs), 0:qw], AF.Exp)
                        Ops_ = psO.tile([128, 2, 256], F32, tag="T4")
                        dps = psO.tile([128, 256], F32, tag="T3")
                        for bb in bbs:
                            K0 = Q0 + 128 * (bb - 2)
                            kpar = (K0 // CW) & 1
                            kblk = (K0 % CW) // 128
                            lo = max(0, 128 * (bb - 2))
                            first, last = bb == bbs[0], bb == bbs[-1]
                            for o in range(2):
                                nc.tensor.matmul(Ops_[:, o, lo:qw],
                                                 vnt[:, kpar, kblk, o * 128:(o + 1) * 128],
                                                 Pt[:, bb, lo:qw],
                                                 start=(first and o == 0),
                                                 stop=(last and o == 1))
                            nc.tensor.matmul(dps[:, lo:qw], ones_sb[:],
                                             Pt[:, bb, lo:qw],
                                             start=first, stop=last)
                        rec = work.tile([128, 256], F32, tag="rec_ta")
                        recs = work.tile([128, 256], F32, tag="recs_ta")
                        nc.vector.reciprocal_approx_accurate(out=rec[:, 0:qw], in_=dps[:, 0:qw], scratch=recs[:, 0:qw])
                        for o in range(2):
                            nc.vector.tensor_mul(out=dstT[:, o, q0:q0 + qw],
                                                 in0=Ops_[:, o, 0:qw], in1=rec[:, 0:qw])
                        q0 += qw
                # ---- fusion + qkv ----
                fused = work.tile([128, 2, CW], F32R, tag="bufC")
                for o in range(2):
                    ps = pspool.tile([128, CW], F32, tag="T1")
                    for k in range(2):
                        nc.tensor.matmul(ps[:, 0:w], fwT_sb[:, k, o, :], sfT[:, k, 0:w],
                                         start=(k == 0), stop=False)
                        nc.tensor.matmul(ps[:, 0:w], fwT_sb[:, 2 + k, o, :], tfT[:, k, 0:w],
                                         start=False, stop=(k == 1))
                    nc.scalar.activation(fused[:, o, 0:w], ps[:, 0:w], AF.Identity,
                                         bias=fb_sb[:, o:o + 1], scale=1.0)
                for o in range(2):
                    psq = pspool.tile([128, CW], F32, tag="T1")
                    psk = pspool.tile([128, CW], F32, tag="T2")
                    for k in range(2):
                        nc.tensor.matmul(psq[:, 0:w], wqT_sb[:, k, o, :], fused[:, k, 0:w],
                                         start=(k == 0), stop=(k == 1))
                        nc.tensor.matmul(psk[:, 0:w], wkT_sb[:, k, o, :], fused[:, k, 0:w],
                                         start=(k == 0), stop=(k == 1))
                    nc.scalar.activation(qT_all[:, o, l0:l0 + w], psq[:, 0:w], AF.Identity,
                                         bias=bq_sb[:, o:o + 1], scale=1.0)
                    nc.scalar.activation(kT_all[:, o, l0:l0 + w], psk[:, 0:w], AF.Identity,
                                         bias=bk_sb[:, o:o + 1], scale=1.0)
                for bi in range(nblk):
                    psv = pspool.tile([128, 256], F32, tag="T3")
                    for k in range(2):
                        nc.tensor.matmul(psv[:], fused[:, k, bi * 128:(bi + 1) * 128],
                                         wvT_sb[:, k, :], start=(k == 0), stop=(k == 1))
                    nc.vector.tensor_add(out=v_all[:, l0 // 128 + bi, :], in0=psv[:], in1=bvb_sb[:])

            def emit_mha(qlo, qhi):
                for Q0 in range(qlo, qhi, 512):
                    qw = min(512, qhi - Q0)
                    nkv = (Q0 + qw) // 128
                    outps = {}
                    for hp in range(4):          # head pairs (2*hp, 2*hp+1)
                        hg = hp // 2
                        rows = [32 * ((2 * hp) % 4), 32 * ((2 * hp + 1) % 4)]
                        Oh = psO.tile([32, 2, 512], F32, tag="T1", name=f"Oh_{hp}")
                        dh = psO.tile([32, 2, 512], F32, tag="T3", name=f"dh_{hp}")
                        for kb in range(nkv):
                            K0 = kb * 128
                            dlt = K0 - Q0
                            lo = max(0, dlt)
                            first, last = kb == 0, kb == nkv - 1
                            Sps = pspool.tile([128, 2, 512], F32, tag="Sbig")
                            for jj in range(2):
                                nc.tensor.matmul(Sps[:, jj, lo:qw],
                                                 kT_all[rows[jj]:rows[jj] + 32, hg, K0:K0 + 128],
                                                 qT_all[rows[jj]:rows[jj] + 32, hg, Q0 + lo:Q0 + qw],
                                                 start=True, stop=True,
                                                 tile_position=(rows[jj], 0))
                            if dlt >= 0:
                                dwd = min(qw, dlt + 128) - dlt
                                for jj in range(2):
                                    nc.vector.tensor_add(out=Sps[:, jj, dlt:dlt + dwd],
                                                         in0=Sps[:, jj, dlt:dlt + dwd],
                                                         in1=T_sb[:, 0:dwd])
                            Pm = wk3.tile([128, 2, 512], F32R, tag="Pbuf")
                            nc.scalar.activation(Pm[:, :, lo:qw], Sps[:, :, lo:qw], AF.Exp)
                            for jj in range(2):
                                h = 2 * hp + jj
                                nc.tensor.matmul(Oh[:, jj, lo:qw],
                                                 v_all[:, kb, h * 32:h * 32 + 32],
                                                 Pm[:, jj, lo:qw], start=first, stop=last)
                                nc.tensor.matmul(dh[:, jj, lo:qw],
                                                 ones_sb[:, 0:32],
                                                 Pm[:, jj, lo:qw], start=first, stop=last)
                        rec = work.tile([32, 2, 512], F32, tag="rec_ta")
                        recs = work.tile([32, 2, 512], F32, tag="recs_ta")
                        nc.vector.reciprocal_approx_accurate(out=rec[:, :, 0:qw], in_=dh[:, :, 0:qw],
                                                             scratch=recs[:, :, 0:qw])
                        Ohn = work.tile([32, 2, 512], F32R, tag=f"Ohn_{hp}", name=f"Ohn_{hp}")
                        outps[hp] = Ohn
                        nc.vector.tensor_mul(out=Ohn[:, :, 0:qw], in0=Oh[:, :, 0:qw], in1=rec[:, :, 0:qw])
                        if Q0 == 0:
                            nc.vector.tensor_copy(out=Ohn[:, :, 0:1],
                                                  in_=zeros8[0:32, 0:2].unsqueeze(-1))
                    for bi in range(qw // 128):
                        pso = pspool.tile([128, 256], F32, tag="T2")
                        for hp in range(4):
                            for jj in range(2):
                                h = 2 * hp + jj
                                nc.tensor.matmul(pso[:], outps[hp][:, jj, bi * 128:(bi + 1) * 128],
                                                 woT_sb[:, h, :], start=(hp == 0 and jj == 0),
                                                 stop=(hp == 3 and jj == 1))
                        ot = work.tile([128, 256], F32, tag="ot")
                        nc.vector.tensor_add(out=ot[:], in0=pso[:], in1=bob_sb[:])
                        nc.sync.dma_start(outd[Q0 + bi * 128:Q0 + (bi + 1) * 128, :], ot[:])

            def emit_variant(vi):
                lo, hi = ABOUNDS[vi], ABOUNDS[vi + 1]
                l0 = 1024
                while l0 < hi:
                    w = min(CW, hi - l0)
                    emit_chunk(l0, w)
                    l0 += w
                emit_mha(lo, hi)

            emit_chunk(0, CW)
            emit_chunk(CW, CW)

            if force_variant is not None:
                emit_variant(force_variant)
            else:
                pid = nc.partition_id()
                with tc.If(pid < 2) as c0:
                    emit_variant(0)
                with c0.Else():
                    with tc.If(pid < 4) as c1:
                        emit_variant(1)
                    with c1.Else():
                        with tc.If(pid < 6) as c2:
                            emit_variant(2)
                        with c2.Else():
                            emit_variant(3)
    nc.finalize()
    return nc


_CACHE = {}


def kernel(**inputs):
    from concourse.bass_utils import run_bass_kernel_spmd
    shared, xT = _host_prep(inputs)
    if "nc" not in _CACHE:
        _CACHE["nc"] = _build()
    nc = _CACHE["nc"]
    in_maps = []
    for core in range(8):
        b = core & 1
        m = dict(shared)
        m["xT"] = np.ascontiguousarray(xT[b])
        in_maps.append(m)
    res = run_bass_kernel_spmd(nc, in_maps, core_ids=list(range(8)))
    out = np.zeros((B, L, D), np.float32)
    for core in range(8):
        b = core & 1
        vi = core // 2
        lo, hi = ABOUNDS[vi], ABOUNDS[vi + 1]
        out[b, lo:hi, :] = res.results[core]["out"][lo:hi, :]
    return out

